# revision 1
# baseline (speedup 1.0000x reference)
"""Trainium2 Bass kernel for nn_InterpreMol_55877524521515.

6-layer post-norm transformer encoder, B=64 molecules, S=255(+CLS)=256,
D=512, H=8 heads, FF=2048, plus a 2-layer head on the CLS token.

Strategy: data-parallel over batch (8 molecules per NeuronCore, 8 cores).
All activations live in SBUF transposed ([D, seq]) as float32r (TF32-like,
full-speed PE matmuls). Weights are SBUF-resident per layer, streamed via a
For_i hardware loop over layers with register-indexed DRAM offsets.
Attention runs in transposed orientation (scoresT[k,q] = k @ qT) so the
attn@V matmul needs no transposes; the softmax denominator comes from a
ones-augmented V operand, and per-query normalization is applied via a K=1
broadcast matmul. Edge bias (+ key padding mask as -30000) is pre-transposed
on the host to fp16 [mol, head, k, q] and streamed per head.
"""
import sys

sys.path.insert(0, "/opt/trn_rl_repo")

import numpy as np

import concourse.bass as bass
import concourse.tile as tile
from concourse import bacc, mybir
from concourse.bass import ds, ts
from concourse.bass_utils import run_bass_kernel_spmd

F32 = mybir.dt.float32
F32R = mybir.dt.float32r
F16 = mybir.dt.float16
AF = mybir.ActivationFunctionType
OP = mybir.AluOpType

B, S, D, H, L, FF, HID = 64, 255, 512, 8, 6, 2048, 256
S1 = 256          # seq with CLS
BL = 8            # molecules per core
DK = D // H       # 64
NCORE = 8
EPS = 1e-5
NEG = -30000.0    # masked-key bias (fp16-safe; exp underflows to 0 in f32)

# pp param-pack column map (per layer, [128, 48]):
#  0:4 bq*0.125 | 4:8 bk | 8:12 bo | 12:28 b1 | 28:32 b2
#  32:36 ln1_g | 36:40 ln1_b | 40:44 ln2_g | 44:48 ln2_b
PPC = 48


def build_program(reps=1, hoist_w=False):
    nc = bacc.Bacc("TRN2", target_bir_lowering=False, debug=False)

    x0t_d = nc.dram_tensor("x0t", [D, BL * S1], F32R, kind="ExternalInput")
    bias_d = nc.dram_tensor("biast", [BL, H, S1, S1], F16, kind="ExternalInput")
    wq_d = nc.dram_tensor("wq", [L * D, D], F32R, kind="ExternalInput")
    wk_d = nc.dram_tensor("wk", [L * D, D], F32R, kind="ExternalInput")
    wv_d = nc.dram_tensor("wv", [L * D, D], F32R, kind="ExternalInput")
    wo_d = nc.dram_tensor("wo", [L * D, D], F32R, kind="ExternalInput")
    w1_d = nc.dram_tensor("w1", [L * D, FF], F32R, kind="ExternalInput")
    w2_d = nc.dram_tensor("w2", [L * FF, D], F32R, kind="ExternalInput")
    pp_d = nc.dram_tensor("pp", [L * 128, PPC], F32, kind="ExternalInput")
    pv_d = nc.dram_tensor("pv", [L, D], F32R, kind="ExternalInput")
    hw1_d = nc.dram_tensor("hw1", [D, HID], F32R, kind="ExternalInput")
    hb1_d = nc.dram_tensor("hb1", [128, 2], F32, kind="ExternalInput")
    hw2_d = nc.dram_tensor("hw2", [128, 2], F32R, kind="ExternalInput")
    hb2_d = nc.dram_tensor("hb2", [1, 1], F32, kind="ExternalInput")
    out_d = nc.dram_tensor("out", [1, BL], F32, kind="ExternalOutput")

    with tile.TileContext(nc) as tc:
        with tc.tile_pool(name="cst", bufs=1) as cst, \
             tc.tile_pool(name="qtp", bufs=5) as qtp, \
             tc.tile_pool(name="ktp", bufs=5) as ktp, \
             tc.tile_pool(name="vgp", bufs=3) as vgp, \
             tc.tile_pool(name="scp", bufs=2) as scp, \
             tc.tile_pool(name="exp_", bufs=3) as exp_, \
             tc.tile_pool(name="atp", bufs=7) as atp, \
             tc.tile_pool(name="xap", bufs=4) as xap, \
             tc.tile_pool(name="xlp", bufs=5) as xlp, \
             tc.tile_pool(name="htp", bufs=16) as htp, \
             tc.tile_pool(name="sqp", bufs=1) as sqp, \
             tc.tile_pool(name="lnp", bufs=4) as lnp, \
             tc.tile_pool(name="bcp", bufs=2) as bcp, \
             tc.tile_pool(name="rcp", bufs=2) as rcp, \
             tc.tile_pool(name="btp", bufs=4) as btp, \
             tc.tile_pool(name="psb", bufs=3, space="PSUM") as psb, \
             tc.tile_pool(name="pss", bufs=2, space="PSUM") as pss, \
             tc.tile_pool(name="psa", bufs=3, space="PSUM") as psa:

            # ---- static tiles -------------------------------------------
            xres = [[cst.tile([128, S1], F32R, name=f"xres_{kt}_{m}")
                     for m in range(BL)] for kt in range(4)]
            wq_sb = [cst.tile([128, D], F32R, name=f"wq_sb{kt}") for kt in range(4)]
            wk_sb = [cst.tile([128, D], F32R, name=f"wk_sb{kt}") for kt in range(4)]
            wv_sb = [cst.tile([128, D], F32R, name=f"wv_sb{kt}") for kt in range(4)]
            wo_sb = [cst.tile([128, D], F32R, name=f"wo_sb{kt}") for kt in range(4)]
            w1_sb = [cst.tile([128, FF], F32R, name=f"w1_sb{kt}") for kt in range(4)]
            w2_sb = [cst.tile([128, D], F32R, name=f"w2_sb{kt}") for kt in range(16)]
            pp_sb = cst.tile([128, PPC], F32, name="pp_sb")
            pv_sb = cst.tile([1, D], F32R, name="pv_sb")
            ones_inv = cst.tile([128, 128], F32R, name="ones_inv")   # 1/512
            ones_r = cst.tile([1, 128], F32R, name="ones_r")         # 1.0
            ones8 = cst.tile([128, 8], F32R, name="ones8")           # 1.0
            hw1_sb = [cst.tile([128, HID], F32R, name=f"hw1_sb{kt}") for kt in range(4)]
            hb1_sb = cst.tile([128, 2], F32, name="hb1_sb")
            hw2_sb = cst.tile([128, 2], F32R, name="hw2_sb")
            hb2_sb = cst.tile([1, 1], F32, name="hb2_sb")
            cls_sb = [cst.tile([128, BL], F32R, name=f"cls_sb{kt}") for kt in range(4)]
            h_sb = [cst.tile([128, BL], F32R, name=f"h_sb{mt}") for mt in range(2)]
            out_sb = cst.tile([1, BL], F32, name="out_sb")
            cinit = cst.tile([128, 128], F32, name="cinit")

            # ---- constants ----------------------------------------------
            nc.vector.memset(cinit[:], 1.0 / D)
            nc.vector.tensor_copy(ones_inv[:], cinit[:])
            nc.vector.memset(cinit[:], 1.0)
            nc.vector.tensor_copy(ones_r[:], cinit[0:1, :])
            nc.vector.tensor_copy(ones8[:], cinit[:, 0:8])

            # ---- initial loads ------------------------------------------
            for kt in range(4):
                for m in range(BL):
                    nc.sync.dma_start(
                        out=xres[kt][m][:],
                        in_=x0t_d.ap()[kt * 128:(kt + 1) * 128, m * S1:(m + 1) * S1])
            for kt in range(4):
                nc.sync.dma_start(out=hw1_sb[kt][:],
                                  in_=hw1_d.ap()[kt * 128:(kt + 1) * 128, :])
            nc.sync.dma_start(out=hb1_sb[:], in_=hb1_d.ap())
            nc.sync.dma_start(out=hw2_sb[:], in_=hw2_d.ap())
            nc.sync.dma_start(out=hb2_sb[:], in_=hb2_d.ap())

            # ---- one transformer layer (emitted once, looped) -----------
            def load_weights(iv):
                for kt in range(4):
                    nc.sync.dma_start(out=wq_sb[kt][:],
                                      in_=wq_d.ap()[ds(iv * D + kt * 128, 128), :])
                for kt in range(4):
                    nc.sync.dma_start(out=wk_sb[kt][:],
                                      in_=wk_d.ap()[ds(iv * D + kt * 128, 128), :])
                for kt in range(4):
                    nc.sync.dma_start(out=wv_sb[kt][:],
                                      in_=wv_d.ap()[ds(iv * D + kt * 128, 128), :])
                nc.sync.dma_start(out=pp_sb[:], in_=pp_d.ap()[ds(iv * 128, 128), :])
                nc.sync.dma_start(out=pv_sb[:], in_=pv_d.ap()[ds(iv, 1), :])
                for kt in range(4):
                    nc.gpsimd.dma_start(out=wo_sb[kt][:],
                                        in_=wo_d.ap()[ds(iv * D + kt * 128, 128), :])
                for kt in range(4):
                    nc.gpsimd.dma_start(out=w1_sb[kt][:],
                                        in_=w1_d.ap()[ds(iv * D + kt * 128, 128), :])
                for kt in range(16):
                    nc.gpsimd.dma_start(out=w2_sb[kt][:],
                                        in_=w2_d.ap()[ds(iv * FF + kt * 128, 128), :])

            def layer_body(iv):
                if not hoist_w:
                    load_weights(iv)
                for m in range(BL):
                    # ---- Q^T, K^T projections ([dout,128] x [din,seq]) --
                    qt_t = []
                    kt_t = []
                    for mt in range(4):
                        ps_q = psb.tile([128, 512], F32, name="ps_q", tag="big")
                        for kt in range(4):
                            nc.tensor.matmul(
                                ps_q[:, 0:S1],
                                wq_sb[kt][:, mt * 128:(mt + 1) * 128],
                                xres[kt][m][:],
                                start=(kt == 0), stop=(kt == 3))
                        q = qtp.tile([128, S1], F32R, name="q")
                        # q_hat = 0.125*(x@Wq) + (0.125*bq)  (bq prescaled on host)
                        nc.scalar.activation(q[:], ps_q[:, 0:S1], AF.Identity,
                                             bias=pp_sb[:, mt:mt + 1], scale=0.125)
                        qt_t.append(q)
                        ps_k = psb.tile([128, 512], F32, name="ps_k", tag="big")
                        for kt in range(4):
                            nc.tensor.matmul(
                                ps_k[:, 0:S1],
                                wk_sb[kt][:, mt * 128:(mt + 1) * 128],
                                xres[kt][m][:],
                                start=(kt == 0), stop=(kt == 3))
                        k = ktp.tile([128, S1], F32R, name="k")
                        nc.scalar.activation(k[:], ps_k[:, 0:S1], AF.Identity,
                                             bias=pp_sb[:, 4 + mt:5 + mt], scale=1.0)
                        kt_t.append(k)

                    # ---- V natural ([seq,512]) + ones column ------------
                    vg_t = []
                    for st in range(2):
                        ps_v = psb.tile([128, 512], F32, name="ps_v", tag="big")
                        for kt in range(4):
                            nc.tensor.matmul(
                                ps_v[:],
                                xres[kt][m][:, st * 128:(st + 1) * 128],
                                wv_sb[kt][:],
                                start=(kt == 0), stop=False)
                        nc.tensor.matmul(ps_v[:], ones_r[:], pv_sb[:],
                                         start=False, stop=True)
                        vg = vgp.tile([128, H, DK + 1], F32R, name="vg")
                        nc.vector.tensor_copy(
                            vg[:, :, 0:DK],
                            ps_v[:].rearrange("p (h d) -> p h d", h=H))
                        nc.vector.tensor_copy(
                            vg[:, :, DK:DK + 1],
                            ones8[:].rearrange("p (h o) -> p h o", o=1))
                        vg_t.append(vg)

                    # ---- attention per head -----------------------------
                    at_t = [atp.tile([128, S1], F32R, name="at") for _ in range(4)]
                    for h in range(8):
                        r0 = (h % 2) * 64
                        bt = btp.tile([128, 2, S1], F16, name="bt")
                        nc.sync.dma_start(
                            out=bt[:],
                            in_=bias_d.ap()[m, h].rearrange(
                                "(a p) q -> p a q", p=128))
                        ps_sc = pss.tile([128, 2, S1], F32, name="ps_sc", tag="sc")
                        for st in range(2):
                            nc.tensor.matmul(
                                ps_sc[:, st, :],
                                kt_t[h // 2][r0:r0 + 64, st * 128:(st + 1) * 128],
                                qt_t[h // 2][r0:r0 + 64, :],
                                start=True, stop=True)
                        sc = scp.tile([128, 2, S1], F32, name="sc")
                        nc.vector.tensor_add(sc[:], ps_sc[:], bt[:])
                        ex = exp_.tile([128, 2, S1], F32R, name="ex")
                        nc.scalar.activation(ex[:], sc[:], AF.Exp)
                        ps_av = psa.tile([128, S1], F32, name="ps_av", tag="av")
                        for st in range(2):
                            nc.tensor.matmul(
                                ps_av[0:DK + 1, :],
                                vg_t[st][:, h, :],
                                ex[:, st, :],
                                start=(st == 0), stop=(st == 1))
                        rc = rcp.tile([1, S1], F32R, name="rc")
                        with nc.allow_low_precision(reason="softmax recip bcast"):
                            nc.vector.reciprocal(rc[:], ps_av[DK:DK + 1, :])
                        ps_bc = psa.tile([64, S1], F32, name="ps_bc", tag="av")
                        nc.tensor.matmul(ps_bc[:], ones_r[0:1, 0:64], rc[:],
                                         start=True, stop=True)
                        bc = bcp.tile([64, S1], F32, name="bc")
                        nc.scalar.copy(bc[:], ps_bc[:])
                        nc.vector.tensor_mul(
                            at_t[h // 2][r0:r0 + 64, :], ps_av[0:DK, :], bc[:])

                    # ---- out proj + residual + LN1 ----------------------
                    xa_t = []
                    for mt in range(4):
                        ps_o = psb.tile([128, 512], F32, name="ps_o", tag="big")
                        for kt in range(4):
                            nc.tensor.matmul(
                                ps_o[:, 0:S1],
                                wo_sb[kt][:, mt * 128:(mt + 1) * 128],
                                at_t[kt][:],
                                start=(kt == 0), stop=(kt == 3))
                        xa = xap.tile([128, S1], F32R, name="xa")
                        nc.vector.scalar_tensor_tensor(
                            xa[:], ps_o[:, 0:S1], pp_sb[:, 8 + mt:9 + mt],
                            xres[mt][m][:], op0=OP.add, op1=OP.add)
                        xa_t.append(xa)

                    xl_t = layer_norm(nc, tc, psb, sqp, lnp, xlp, xa_t, pp_sb,
                                      ones_inv, 32, None, None)

                    # ---- FFN ------------------------------------------
                    ht_t = []
                    for fb in range(16):
                        ps_f = psb.tile([128, 512], F32, name="ps_f", tag="big")
                        for kt in range(4):
                            nc.tensor.matmul(
                                ps_f[:, 0:S1],
                                w1_sb[kt][:, fb * 128:(fb + 1) * 128],
                                xl_t[kt][:],
                                start=(kt == 0), stop=(kt == 3))
                        ht = htp.tile([128, S1], F32R, name="ht")
                        nc.scalar.activation(ht[:], ps_f[:, 0:S1], AF.Gelu,
                                             bias=pp_sb[:, 12 + fb:13 + fb])
                        ht_t.append(ht)

                    xb_t = []
                    for mt in range(4):
                        ps_g = psb.tile([128, 512], F32, name="ps_g", tag="big")
                        for kt in range(16):
                            nc.tensor.matmul(
                                ps_g[:, 0:S1],
                                w2_sb[kt][:, mt * 128:(mt + 1) * 128],
                                ht_t[kt][:],
                                start=(kt == 0), stop=(kt == 15))
                        xb = xap.tile([128, S1], F32R, name="xb", tag="xa")
                        nc.vector.scalar_tensor_tensor(
                            xb[:], ps_g[:, 0:S1], pp_sb[:, 28 + mt:29 + mt],
                            xl_t[mt][:], op0=OP.add, op1=OP.add)
                        xb_t.append(xb)

                    layer_norm(nc, tc, psb, sqp, lnp, xlp, xb_t, pp_sb,
                               ones_inv, 40, xres, m)

            if hoist_w:
                load_weights(0)
            if reps > 1:
                with tc.For_i(0, reps, 1) as rv:
                    with tc.For_i(0, L, 1) as iv:
                        layer_body(iv)
            else:
                with tc.For_i(0, L, 1) as iv:
                    layer_body(iv)

            # ---- head on CLS tokens -------------------------------------
            for kt in range(4):
                for m in range(BL):
                    nc.vector.tensor_copy(cls_sb[kt][:, m:m + 1],
                                          xres[kt][m][:, 0:1])
            for mt in range(2):
                ps_h = psb.tile([128, 512], F32, name="ps_h", tag="big")
                for kt in range(4):
                    nc.tensor.matmul(
                        ps_h[:, 0:BL],
                        hw1_sb[kt][:, mt * 128:(mt + 1) * 128],
                        cls_sb[kt][:],
                        start=(kt == 0), stop=(kt == 3))
                nc.scalar.activation(h_sb[mt][:], ps_h[:, 0:BL], AF.Gelu,
                                     bias=hb1_sb[:, mt:mt + 1])
            ps_out = psb.tile([128, 512], F32, name="ps_out", tag="big")
            for mt in range(2):
                nc.tensor.matmul(ps_out[0:1, 0:BL], hw2_sb[:, mt:mt + 1],
                                 h_sb[mt][:], start=(mt == 0), stop=(mt == 1))
            nc.scalar.activation(out_sb[:], ps_out[0:1, 0:BL], AF.Identity,
                                 bias=hb2_sb[0:1, 0:1])
            nc.sync.dma_start(out=out_d.ap(), in_=out_sb[:])

    nc.compile()
    return nc


def layer_norm(nc, tc, psb, sqp, lnp, xlp, x_t, pp_sb, ones_inv, gcol,
               xres, mres):
    """LN over the partition (D) dim of 4 x [128, S1] tiles.

    If xres is None, writes 4 fresh xlp tiles and returns them; else writes
    into xres[kt][mres] (persistent residual stream).
    """
    ps_st = psb.tile([128, 512], F32, name="ps_st", tag="big")
    for kt in range(4):
        nc.tensor.matmul(ps_st[:, 0:S1], ones_inv[:], x_t[kt][:],
                         start=(kt == 0), stop=(kt == 3))
    for kt in range(4):
        sq = sqp.tile([128, S1], F32R, name="sq")
        nc.scalar.activation(sq[:], x_t[kt][:], AF.Square)
        nc.tensor.matmul(ps_st[:, S1:2 * S1], ones_inv[:], sq[:],
                         start=(kt == 0), stop=(kt == 3))
    m2 = lnp.tile([128, S1], F32, name="m2", tag="ln")
    nc.scalar.activation(m2[:], ps_st[:, 0:S1], AF.Square)
    var = lnp.tile([128, S1], F32, name="var", tag="ln")
    nc.vector.scalar_tensor_tensor(var[:], ps_st[:, S1:2 * S1], EPS, m2[:],
                                   op0=OP.add, op1=OP.subtract)
    std = lnp.tile([128, S1], F32, name="std", tag="ln")
    nc.scalar.activation(std[:], var[:], AF.Sqrt)
    rstd = lnp.tile([128, S1], F32, name="rstd", tag="ln")
    nc.vector.reciprocal(rstd[:], std[:])
    outs = []
    for kt in range(4):
        cen = lnp.tile([128, S1], F32, name="cen", tag="ln")
        nc.vector.tensor_sub(cen[:], x_t[kt][:], ps_st[:, 0:S1])
        nrm = lnp.tile([128, S1], F32, name="nrm", tag="ln")
        nc.vector.tensor_mul(nrm[:], cen[:], rstd[:])
        if xres is None:
            o = xlp.tile([128, S1], F32R, name="xl")
            outs.append(o)
            dst = o[:]
        else:
            dst = xres[kt][mres][:]
        nc.scalar.activation(dst, nrm[:], AF.Identity,
                             bias=pp_sb[:, gcol + 4 + kt:gcol + 5 + kt],
                             scale=pp_sb[:, gcol + kt:gcol + 1 + kt])
    return outs


_CACHE = {}


def _get_program(reps):
    if reps not in _CACHE:
        _CACHE[reps] = build_program(reps)
    return _CACHE[reps]


def prep_inputs(atom_emb, edge_bias, key_padding_mask, cls_token, Wq, bq, Wk,
                bk, Wv, bv, Wo, bo, ln1_g, ln1_b, W1, b1, W2, b2, ln2_g,
                ln2_b, head_W1, head_b1, head_W2, head_b2):
    f32 = np.float32
    atom_emb = np.asarray(atom_emb, f32)
    cls_token = np.asarray(cls_token, f32)
    x0 = np.concatenate(
        [np.broadcast_to(cls_token, (B, 1, D)), atom_emb], axis=1)  # [B,S1,D]

    # biasT[b,h,k,q] = edge_bias[b,q-1,k-1,h]; masked key rows -> NEG
    bt = np.zeros((B, H, S1, S1), np.float16)
    eb = np.asarray(edge_bias, f32).transpose(0, 3, 2, 1)  # [b,h,j(k),i(q)]
    bt[:, :, 1:, 1:] = eb.astype(np.float16)
    km = np.asarray(key_padding_mask, bool)
    bi, ki = np.nonzero(km)
    bt[bi, :, ki + 1, :] = np.float16(NEG)

    def seg(x):  # [L, dim] -> [L, dim//128, 128] -> [L, 128, dim//128]
        x = np.asarray(x, f32)
        return x.reshape(L, -1, 128).transpose(0, 2, 1)

    pp = np.zeros((L, 128, PPC), f32)
    pp[:, :, 0:4] = seg(np.asarray(bq, f32) * 0.125)
    pp[:, :, 4:8] = seg(bk)
    pp[:, :, 8:12] = seg(bo)
    pp[:, :, 12:28] = seg(b1)
    pp[:, :, 28:32] = seg(b2)
    pp[:, :, 32:36] = seg(ln1_g)
    pp[:, :, 36:40] = seg(ln1_b)
    pp[:, :, 40:44] = seg(ln2_g)
    pp[:, :, 44:48] = seg(ln2_b)

    shared = {
        "wq": np.ascontiguousarray(np.asarray(Wq, f32).reshape(L * D, D)),
        "wk": np.ascontiguousarray(np.asarray(Wk, f32).reshape(L * D, D)),
        "wv": np.ascontiguousarray(np.asarray(Wv, f32).reshape(L * D, D)),
        "wo": np.ascontiguousarray(np.asarray(Wo, f32).reshape(L * D, D)),
        "w1": np.ascontiguousarray(np.asarray(W1, f32).reshape(L * D, FF)),
        "w2": np.ascontiguousarray(np.asarray(W2, f32).reshape(L * FF, D)),
        "pp": np.ascontiguousarray(pp.reshape(L * 128, PPC)),
        "pv": np.ascontiguousarray(np.asarray(bv, f32)),
        "hw1": np.ascontiguousarray(np.asarray(head_W1, f32)),
        "hb1": np.ascontiguousarray(
            np.asarray(head_b1, f32).reshape(2, 128).T),
        "hw2": np.ascontiguousarray(
            np.asarray(head_W2, f32).reshape(2, 128).T),
        "hb2": np.asarray(head_b2, f32).reshape(1, 1),
    }
    in_maps = []
    for c in range(NCORE):
        sl = slice(c * BL, (c + 1) * BL)
        x0t = np.ascontiguousarray(
            x0[sl].transpose(2, 0, 1).reshape(D, BL * S1))
        in_maps.append({"x0t": x0t, "biast": np.ascontiguousarray(bt[sl]),
                        **shared})
    return in_maps


def run(in_maps, reps=1):
    nc = _get_program(reps)
    res = run_bass_kernel_spmd(nc, in_maps, list(range(NCORE)))
    out = np.concatenate([res.results[c]["out"].reshape(BL, 1)
                          for c in range(NCORE)], axis=0)
    return out


def kernel(**inputs) -> np.ndarray:
    in_maps = prep_inputs(**inputs)
    return run(in_maps, reps=1)

